# revision 50
# baseline (speedup 1.0000x reference)
"""Trainium2 Bass kernel for nn_AtomicKANLayer.

Math: y[b,o] = sum_{i,d} fupn((x[b,i]-centers[d])*compression[d]) * coeffs[i,o,d]
with fupn the atomic basis function evaluated via its (truncated) Fourier
series.  With theta_k = t_k*comp and phi_kd = t_k*comp*c_d:

  fupn(z_d) = mask_d * (0.5 + sum_k c_k [cos(theta_k x)cos(phi_kd) +
                                          sin(theta_k x)sin(phi_kd)]) / a

i.e. a dense matmul over per-element trig features cos/sin(theta_k x).  c_15
is exactly 0 and the tail beyond k=14 sums to 3.4e-5, so NK=14 terms suffice;
28 trig rows per quarter * 4 quarters pack the 4096 per-core elements into
112 partitions at 1024 columns, halving every elementwise pass vs a 2-way
split.  The support mask (and the DC 0.5 term) need no device compute at
all: the mask depends only on x, so the host ships it as an fp16 0/1 tensor
and a single fused DVE op per wave does  basis = (s + dc) * mask  straight
out of PSUM.  The final 1/CO_SCALE is folded into the W columns.

Device pipeline per core (data-parallel over batch, 32 rows of B=256 each):
  1. q[p,c] = theta_p/2pi * x + off_p  as an exact split-fp16 matmul
     (th_h*xh + th_h*xl + th_l*xh, 13-row contraction, fp32 PSUM)     [PE]
  2. f = q - round(q) (fp32 magic-constant trick: ts add/sub, tt sub) [DVE]
  3. trig[0:112] = Sin(2pi f) -> fp16 (ACT free scale)                [ACT]
  4. s = trig-chunk.T @ W_h (fp16 K=112, 33 cols) per b               [PE]
  5. basis = (s + dc) * mask: fused STT per wave                      [DVE]
  6. y = sum_d basis_d.T @ coeffs_d (33 fp16 matmuls, fp32 PSUM)      [PE]
  7. y PSUM -> SBUF copy (ACT), DMA out                               [ACT]
All input DMAs ride one sync-HWDGE FIFO (the DMA engines serve the
first-ringing queue until it drains, so cross-queue splits starve someone):
ph first (q gate), then W (s-matmul gate), the fp8 mask, and the 2.1MB
coefficient stream in 4 d-chunks consumed in order by step 6.  Warm-up
matmuls on const/loaded tiles keep the tensor clock ramping (0.65/1.2/2.4
GHz p-states) through every pipeline stall, and two tiny warmer DMAs keep
the queue hot for the output transfer.
"""
import sys

sys.path.insert(0, "/opt/trn_rl_repo")

import numpy as np

BF16 = np.float16  # half precision for PE operands (fp32 PSUM accumulate)
B, I, O, D = 256, 128, 256, 33
NCORES = 8
BLOC = B // NCORES          # 32 batch rows per core
ROWS = BLOC * I             # 4096 flattened (b, i) elements per core
NQ = 4                      # partition-packing quarters
QCOLS = ROWS // NQ          # 1024 columns per quarter
NK = 14                     # Fourier terms kept (c_15 == 0 exactly)
QROWS = 2 * NK              # cos/sin rows per quarter
KTRIG = NQ * QROWS          # 112 trig partitions
N_ORDER, NPROD = 1, 10
A_SUP = (N_ORDER + 2) / 2.0  # support half-width a = 1.5
MAGIC_A, MAGIC_B = 49152.0, 256.0  # fp16-exact pair, product 1.5*2^23
TWO_PI = float(2 * np.pi)
CO_DMA_CHUNKS = 4
CO_SCALE = 256.0            # lift fp16 coeffs out of subnormal range
DC = float(0.5 / (A_SUP * CO_SCALE))  # series DC term, added in the STT
# STT waves: (b0, b1); each wave lives in its own PSUM bank (<=15 b's)
WAVES = ((0, 15), (15, 26), (26, 32))
# PE warm-up dummies: the tensor clock ramps 0.65->1.2->2.4GHz with
# sustained execution; tiny matmuls fill every idle window so the final
# contraction runs fully ramped.  (pre-ph, post-q, mid-wave, pre-y)
DUMMIES = (9,)
BIG_DUMMIES = (4, 2, 6)   # post-q, mid-wave, pre-y (y-shaped, ramp the clock)

_PROG = None


def _build_program():
    import concourse.bacc as bacc
    import concourse.tile as tile
    from concourse import mybir

    f32 = mybir.dt.float32
    f16 = mybir.dt.float16
    f8 = mybir.dt.float8e4
    Alu = mybir.AluOpType
    Act = mybir.ActivationFunctionType

    nc = bacc.Bacc("TRN2", target_bir_lowering=False, debug=False,
                   num_devices=NCORES)
    ph_d = nc.dram_tensor("ph", [28, QCOLS + 128], f16, kind="ExternalInput")
    w_d = nc.dram_tensor("w", [KTRIG, NQ * D], f16, kind="ExternalInput")
    mk_d = nc.dram_tensor("mk", [I, BLOC * D], f8, kind="ExternalInput")
    co_d = nc.dram_tensor("co", [I, D * O], f16, kind="ExternalInput")
    y_d = nc.dram_tensor("y_s", [BLOC, O], f32, kind="ExternalOutput")

    with tile.TileContext(nc) as tc:
        with (
            tc.tile_pool(name="const", bufs=1) as cpool,
            tc.tile_pool(name="work", bufs=2) as wpool,
            tc.tile_pool(name="qp", bufs=2, space="PSUM") as qpool,
            tc.tile_pool(name="sza", bufs=1, space="PSUM") as szap,
            tc.tile_pool(name="szb", bufs=1, space="PSUM") as szbp,
            tc.tile_pool(name="szc", bufs=1, space="PSUM") as szcp,
            tc.tile_pool(name="yp", bufs=1, space="PSUM") as ypool,
            tc.tile_pool(name="dum", bufs=1, space="PSUM") as dumpool,
        ):
            from concourse.tile_rust import add_dep_helper

            # one FIFO: ph (padded to 16 rows -- one descriptor per DMA
            # engine for the fastest completion credit), then w, mask, and
            # the coefficient chunks
            ph_t = cpool.tile([28, QCOLS + 128], f16)
            with tc.high_priority():
                nc.scalar.dma_start(ph_t[:], ph_d.ap()[:])
            co_t = cpool.tile([I, D * O], f16)
            dper = (D + CO_DMA_CHUNKS - 1) // CO_DMA_CHUNKS

            def co_dma(c):
                d0, d1 = c * dper, min(D, (c + 1) * dper)
                nc.sync.dma_start(co_t[:, O * d0:O * d1],
                                  co_d.ap()[:, O * d0:O * d1])

            w_t = cpool.tile([KTRIG, NQ * D], f16)
            nc.sync.dma_start(w_t[:], w_d.ap()[:])
            mk_t = cpool.tile([I, BLOC * D], f8)
            nc.sync.dma_start(mk_t[:], mk_d.ap()[:])
            for c in range(CO_DMA_CHUNKS):
                co_dma(c)

            uq_v = ph_t[:, 0:QCOLS]
            pq_v = ph_t[:, QCOLS:QCOLS + 128]

            import contextlib

            # PE warm-up dummies: back-to-back 1x1 matmuls on the framework
            # const AP keep the tensor clock ramping through every stall
            cb = nc.const_aps.aps[(mybir.dt.bfloat16, 1.0)]
            cbl = nc.const_aps.tensor(1.0, (128, 128), mybir.dt.bfloat16)
            cbr = nc.const_aps.tensor(1.0, (128, O), mybir.dt.bfloat16)
            dumc = dumpool.tile([128, O], f32)
            pe_last = [None]

            def pe_chain(ins_obj):
                ins = getattr(ins_obj, "ins", ins_obj)
                if pe_last[0] is not None:
                    add_dep_helper(ins, pe_last[0], sync=False,
                                   reason="keep PE in emission order")
                pe_last[0] = ins
                return ins_obj

            # y-shaped dummies (256-col streams off the mask tile): the
            # clock governor ramps on sustained HIGH-utilization matmuls,
            # not on tiny ones, so these pre-heat the final contraction
            dumb = dumpool.tile([32, O], f32)

            def big_dummies(n):
                for _ in range(n):
                    pe_chain(nc.tensor.matmul(dumb[:], ph_t[:, 0:32],
                                              ph_t[:, 0:O],
                                              start=True, stop=True))

            # full-power warm-up before any input lands: broadcast const
            # operands, full 128x128 array, 256-col streams
            for _ in range(DUMMIES[0]):
                pe_chain(nc.tensor.matmul(dumc[:], cbl, cbr,
                                          start=True, stop=True))

            # phase matmul, two 512-column chunks.  The PE accumulates the
            # K dot product sequentially in row order in fp32, so rows
            # [theta-terms, +M, -M, -theta-terms] (M = 1.5*2^23 = 49152*256,
            # fp16-exact) round the running sum to an integer mid-matmul and
            # leave -frac(q) directly in PSUM: the whole magic-number
            # rounding costs zero DVE ops.  Sin(scale=-2pi) reads the PSUM.
            trig = cpool.tile([KTRIG, QCOLS], f16)
            for ch in range(2):
                cs = slice(512 * ch, 512 * (ch + 1))
                prio = tc.high_priority() if ch == 0 else (
                    contextlib.nullcontext())
                with prio:
                    q = qpool.tile([128, 512], f32, tag="q")
                    pe_chain(nc.tensor.matmul(q[:], pq_v, uq_v[:, cs],
                                              start=True, stop=True))
                    nc.scalar.activation(trig[:, cs], q[0:KTRIG, :],
                                         Act.Sin, scale=-TWO_PI)

            # s matmuls: one 33-col matmul per b; quarter h = b%4 picks the
            # W block, column group g = b//4 picks the trig columns
            szA = szap.tile([128, 512], f32)
            szB = szbp.tile([128, 512], f32)
            szC = szcp.tile([128, 512], f32)
            sz_tiles = (szA, szB, szC)

            def sz_slot(b):
                for wv, (b0, b1) in enumerate(WAVES):
                    if b < b1:
                        return sz_tiles[wv], D * (b - b0)
                raise AssertionError

            big_dummies(BIG_DUMMIES[0])
            for b in range(BLOC):
                if b == 16:
                    big_dummies(BIG_DUMMIES[1])
                g, h = divmod(b, NQ)
                t_sz, off = sz_slot(b)
                pe_chain(nc.tensor.matmul(t_sz[:, off:off + D],
                                          trig[:, 128 * g:128 * (g + 1)],
                                          w_t[:, D * h:D * (h + 1)],
                                          start=True, stop=True))

            # basis = (s + dc) * mask, one fused STT per wave
            bas = cpool.tile([I, BLOC * D], f16)
            for wv, (b0, b1) in enumerate(WAVES):
                ncols = D * (b1 - b0)
                nc.vector.scalar_tensor_tensor(
                    bas[:, D * b0:D * b0 + ncols],
                    in0=sz_tiles[wv][:, 0:ncols], scalar=DC,
                    in1=mk_t[:, D * b0:D * b0 + ncols],
                    op0=Alu.add, op1=Alu.mult)

            # final contraction: accumulate over d (lhsT strided over b-major
            # bas: column b at free index b*D + d)
            big_dummies(BIG_DUMMIES[2])
            basb = bas[:].rearrange("p (b c) -> p c b", c=D)
            y_t = ypool.tile([BLOC, O], f32)
            y_mms = []
            for d in range(D):
                y_mms.append(pe_chain(nc.tensor.matmul(
                    y_t[:], basb[:, d, :], co_t[:, O * d:O * (d + 1)],
                    start=(d == 0), stop=(d == D - 1))))
            # wake the sync DMA queue before the output transfer needs it
            wr1 = cpool.tile([16, 8], f16)
            wr2 = cpool.tile([16, 8], f16)
            for wr_t, dmm in ((wr1, y_mms[12]), (wr2, y_mms[26])):
                wi = nc.sync.dma_start(wr_t[:], co_d.ap()[0:16, 0:8])
                add_dep_helper(wi.ins, dmm.ins, sync=True,
                               reason="keep DMA queue warm for the output")
            y_s = cpool.tile([BLOC, O], f32)
            nc.scalar.copy(y_s[:], y_t[:])
            nc.sync.dma_start(y_d.ap()[:], y_s[:])

    nc.compile()
    return nc


def _host_constants(compression, centers):
    comp = np.asarray(compression, np.float64)
    cent = np.asarray(centers, np.float64)
    assert comp.shape == (D,) and cent.shape == (D,)
    assert np.all(comp == comp[0]), "kernel assumes uniform compression"
    cval = comp[0]

    k = np.arange(1, NK + 1, dtype=np.float64)
    t = (np.pi / A_SUP) * k
    sinc = lambda z: np.sinc(z / np.pi)
    c = sinc(t / 2.0) ** N_ORDER
    for j in range(1, NPROD + 1):
        c = c * sinc(t / (2.0 ** j))

    # per-partition phase constants: theta/2pi split into fp16 hi+lo, and
    # the 1/4-turn offset for cos rows.  Partition map: quarter h occupies
    # [28h, 28h+28) = 14 cos rows then 14 sin rows.
    th = np.zeros(128, np.float64)
    off = np.zeros(128, np.float64)
    feat = t * cval / (2 * np.pi)                    # (NK,)
    for h in range(NQ):
        r = QROWS * h
        th[r:r + NK] = feat
        th[r + NK:r + QROWS] = feat
        off[r:r + NK] = 0.25
    th_h = th.astype(BF16).astype(np.float64)
    th_l = (th - th_h).astype(BF16).astype(np.float64)
    pq = np.zeros((28, 128), np.float64)
    for r0, sgn in ((0, 1.0), (15, -1.0)):
        for h in range(NQ):
            r = QROWS * h
            sel = np.zeros(128)
            sel[r:r + QROWS] = 1.0
            pq[r0 + 3 * h + 0] = sgn * th_h * sel
            pq[r0 + 3 * h + 1] = sgn * th_h * sel
            pq[r0 + 3 * h + 2] = sgn * th_l * sel
        pq[r0 + 12] = sgn * off
    pq[13] = MAGIC_A
    pq[14] = -MAGIC_A

    # feature->series weights; block h zeroes the other quarters' rows.
    # Carries the 1/CO_SCALE output scale; DC term added in the STT.
    phi = np.outer(t * cval, cent)                  # (NK, D)
    s_scale = 1.0 / (A_SUP * CO_SCALE)
    w = np.zeros((KTRIG, NQ * D), np.float64)
    for h in range(NQ):
        blk = D * h
        r = QROWS * h
        w[r:r + NK, blk:blk + D] = c[:, None] * np.cos(phi) * s_scale
        w[r + NK:r + QROWS, blk:blk + D] = c[:, None] * np.sin(phi) * s_scale
    return pq.astype(BF16), w.astype(BF16), cval, cent


def _run(inputs, trace=False, **kw):
    global _PROG
    from concourse.bass_utils import run_bass_kernel_spmd

    if _PROG is None:
        _PROG = _build_program()
    nc = _PROG

    x = np.ascontiguousarray(np.asarray(inputs["x"], np.float32))
    coeffs = np.asarray(inputs["atomic_coeffs"], np.float32)
    pq, w, cval, cent = _host_constants(inputs["compression"],
                                        inputs["centers"])
    co = np.ascontiguousarray(
        (coeffs.transpose(0, 2, 1) * CO_SCALE).astype(BF16).reshape(I, D * O))

    in_maps = []
    for cid in range(NCORES):
        xc = x[cid * BLOC:(cid + 1) * BLOC]              # (32, 128)
        # quarter h holds batch rows b == h (mod 4); col = (b//4)*128 + i
        xq = (xc.reshape(BLOC // NQ, NQ, I).transpose(1, 0, 2)
              .reshape(NQ, QCOLS))
        xh = xq.astype(BF16)
        xl = (xq - xh.astype(np.float32)).astype(BF16)
        ph = np.zeros((28, QCOLS + 128), BF16)
        for r0 in (0, 15):
            for h in range(NQ):
                ph[r0 + 3 * h + 0, :QCOLS] = xh[h]
                ph[r0 + 3 * h + 1, :QCOLS] = xl[h]
                ph[r0 + 3 * h + 2, :QCOLS] = xh[h]
            ph[r0 + 12, :QCOLS] = BF16(1.0)
        ph[13, :QCOLS] = BF16(MAGIC_B)
        ph[14, :QCOLS] = BF16(MAGIC_B)
        ph[:, QCOLS:] = pq
        # support mask [i, b*D + d] = |x[b,i] - c_d| * comp <= a, from exact x
        z = (xc.astype(np.float64)[:, :, None] - cent[None, None, :]) * cval
        mk = np.where(np.abs(z) <= A_SUP, np.uint8(0x38),
                      np.uint8(0))                       # e4m3 1.0 / 0.0
        mk = np.ascontiguousarray(mk.transpose(1, 0, 2).reshape(I, BLOC * D))
        in_maps.append({"ph": ph, "w": w, "mk": mk, "co": co})

    res = run_bass_kernel_spmd(nc, in_maps, core_ids=list(range(NCORES)),
                               trace=trace, **kw)
    # device b index = 4*(b//4) + b%4 = original batch row: no reorder
    y = np.concatenate([res.results[c]["y_s"] for c in range(NCORES)], axis=0)
    return y.astype(np.float32, copy=False), res


def kernel(**inputs):
    y, _ = _run(inputs, trace=False)
    return y


# revision 51
# speedup vs baseline: 1.0022x; 1.0022x over previous
"""Trainium2 Bass kernel for nn_AtomicKANLayer.

Math: y[b,o] = sum_{i,d} fupn((x[b,i]-centers[d])*compression[d]) * coeffs[i,o,d]
with fupn the atomic basis function evaluated via its (truncated) Fourier
series.  With theta_k = t_k*comp and phi_kd = t_k*comp*c_d:

  fupn(z_d) = mask_d * (0.5 + sum_k c_k [cos(theta_k x)cos(phi_kd) +
                                          sin(theta_k x)sin(phi_kd)]) / a

i.e. a dense matmul over per-element trig features cos/sin(theta_k x).  c_15
is exactly 0 and the tail beyond k=14 sums to 3.4e-5, so NK=14 terms suffice;
28 trig rows per quarter * 4 quarters pack the 4096 per-core elements into
112 partitions at 1024 columns, halving every elementwise pass vs a 2-way
split.  The support mask (and the DC 0.5 term) need no device compute at
all: the mask depends only on x, so the host ships it as an fp16 0/1 tensor
and a single fused DVE op per wave does  basis = (s + dc) * mask  straight
out of PSUM.  The final 1/CO_SCALE is folded into the W columns.

Device pipeline per core (data-parallel over batch, 32 rows of B=256 each):
  1. q[p,c] = theta_p/2pi * x + off_p  as an exact split-fp16 matmul
     (th_h*xh + th_h*xl + th_l*xh, 13-row contraction, fp32 PSUM)     [PE]
  2. f = q - round(q) (fp32 magic-constant trick: ts add/sub, tt sub) [DVE]
  3. trig[0:112] = Sin(2pi f) -> fp16 (ACT free scale)                [ACT]
  4. s = trig-chunk.T @ W_h (fp16 K=112, 33 cols) per b               [PE]
  5. basis = (s + dc) * mask: fused STT per wave                      [DVE]
  6. y = sum_d basis_d.T @ coeffs_d (33 fp16 matmuls, fp32 PSUM)      [PE]
  7. y PSUM -> SBUF copy (ACT), DMA out                               [ACT]
All input DMAs ride one sync-HWDGE FIFO (the DMA engines serve the
first-ringing queue until it drains, so cross-queue splits starve someone):
ph first (q gate), then W (s-matmul gate), the fp8 mask, and the 2.1MB
coefficient stream in 4 d-chunks consumed in order by step 6.  Warm-up
matmuls on const/loaded tiles keep the tensor clock ramping (0.65/1.2/2.4
GHz p-states) through every pipeline stall, and two tiny warmer DMAs keep
the queue hot for the output transfer.
"""
import sys

sys.path.insert(0, "/opt/trn_rl_repo")

import numpy as np

BF16 = np.float16  # half precision for PE operands (fp32 PSUM accumulate)
B, I, O, D = 256, 128, 256, 33
NCORES = 8
BLOC = B // NCORES          # 32 batch rows per core
ROWS = BLOC * I             # 4096 flattened (b, i) elements per core
NQ = 4                      # partition-packing quarters
QCOLS = ROWS // NQ          # 1024 columns per quarter
NK = 14                     # Fourier terms kept (c_15 == 0 exactly)
QROWS = 2 * NK              # cos/sin rows per quarter
KTRIG = NQ * QROWS          # 112 trig partitions
N_ORDER, NPROD = 1, 10
A_SUP = (N_ORDER + 2) / 2.0  # support half-width a = 1.5
MAGIC_A, MAGIC_B = 49152.0, 256.0  # fp16-exact pair, product 1.5*2^23
TWO_PI = float(2 * np.pi)
CO_DMA_CHUNKS = 4
CO_SCALE = 256.0            # lift fp16 coeffs out of subnormal range
DC = float(0.5 / (A_SUP * CO_SCALE))  # series DC term, added in the STT
# STT waves: (b0, b1); each wave lives in its own PSUM bank (<=15 b's)
WAVES = ((0, 15), (15, 26), (26, 32))
# PE warm-up dummies: the tensor clock ramps 0.65->1.2->2.4GHz with
# sustained execution; tiny matmuls fill every idle window so the final
# contraction runs fully ramped.  (pre-ph, post-q, mid-wave, pre-y)
DUMMIES = (11,)
BIG_DUMMIES = (4, 2, 6)   # post-q, mid-wave, pre-y (y-shaped, ramp the clock)

_PROG = None


def _build_program():
    import concourse.bacc as bacc
    import concourse.tile as tile
    from concourse import mybir

    f32 = mybir.dt.float32
    f16 = mybir.dt.float16
    f8 = mybir.dt.float8e4
    Alu = mybir.AluOpType
    Act = mybir.ActivationFunctionType

    nc = bacc.Bacc("TRN2", target_bir_lowering=False, debug=False,
                   num_devices=NCORES)
    ph_d = nc.dram_tensor("ph", [28, QCOLS + 128], f16, kind="ExternalInput")
    w_d = nc.dram_tensor("w", [KTRIG, NQ * D], f16, kind="ExternalInput")
    mk_d = nc.dram_tensor("mk", [I, BLOC * D], f8, kind="ExternalInput")
    co_d = nc.dram_tensor("co", [I, D * O], f16, kind="ExternalInput")
    y_d = nc.dram_tensor("y_s", [BLOC, O], f32, kind="ExternalOutput")

    with tile.TileContext(nc) as tc:
        with (
            tc.tile_pool(name="const", bufs=1) as cpool,
            tc.tile_pool(name="work", bufs=2) as wpool,
            tc.tile_pool(name="qp", bufs=2, space="PSUM") as qpool,
            tc.tile_pool(name="sza", bufs=1, space="PSUM") as szap,
            tc.tile_pool(name="szb", bufs=1, space="PSUM") as szbp,
            tc.tile_pool(name="szc", bufs=1, space="PSUM") as szcp,
            tc.tile_pool(name="yp", bufs=1, space="PSUM") as ypool,
            tc.tile_pool(name="dum", bufs=1, space="PSUM") as dumpool,
        ):
            from concourse.tile_rust import add_dep_helper

            # one FIFO: ph (padded to 16 rows -- one descriptor per DMA
            # engine for the fastest completion credit), then w, mask, and
            # the coefficient chunks
            ph_t = cpool.tile([28, QCOLS + 128], f16)
            with tc.high_priority():
                nc.scalar.dma_start(ph_t[:], ph_d.ap()[:])
            co_t = cpool.tile([I, D * O], f16)
            dper = (D + CO_DMA_CHUNKS - 1) // CO_DMA_CHUNKS

            def co_dma(c):
                d0, d1 = c * dper, min(D, (c + 1) * dper)
                nc.sync.dma_start(co_t[:, O * d0:O * d1],
                                  co_d.ap()[:, O * d0:O * d1])

            w_t = cpool.tile([KTRIG, NQ * D], f16)
            nc.sync.dma_start(w_t[:], w_d.ap()[:])
            mk_t = cpool.tile([I, BLOC * D], f8)
            nc.sync.dma_start(mk_t[:], mk_d.ap()[:])
            for c in range(CO_DMA_CHUNKS):
                co_dma(c)

            uq_v = ph_t[:, 0:QCOLS]
            pq_v = ph_t[:, QCOLS:QCOLS + 128]

            import contextlib

            # PE warm-up dummies: back-to-back 1x1 matmuls on the framework
            # const AP keep the tensor clock ramping through every stall
            cb = nc.const_aps.aps[(mybir.dt.bfloat16, 1.0)]
            cbl = nc.const_aps.tensor(1.0, (128, 128), mybir.dt.bfloat16)
            cbr = nc.const_aps.tensor(1.0, (128, O), mybir.dt.bfloat16)
            dumc = dumpool.tile([128, O], f32)
            pe_last = [None]

            def pe_chain(ins_obj):
                ins = getattr(ins_obj, "ins", ins_obj)
                if pe_last[0] is not None:
                    add_dep_helper(ins, pe_last[0], sync=False,
                                   reason="keep PE in emission order")
                pe_last[0] = ins
                return ins_obj

            # y-shaped dummies (256-col streams off the mask tile): the
            # clock governor ramps on sustained HIGH-utilization matmuls,
            # not on tiny ones, so these pre-heat the final contraction
            dumb = dumpool.tile([32, O], f32)

            def big_dummies(n):
                for _ in range(n):
                    pe_chain(nc.tensor.matmul(dumb[:], ph_t[:, 0:32],
                                              ph_t[:, 0:O],
                                              start=True, stop=True))

            # full-power warm-up before any input lands: broadcast const
            # operands, full 128x128 array, 256-col streams
            for _ in range(DUMMIES[0]):
                pe_chain(nc.tensor.matmul(dumc[:], cbl, cbr,
                                          start=True, stop=True))

            # phase matmul, two 512-column chunks.  The PE accumulates the
            # K dot product sequentially in row order in fp32, so rows
            # [theta-terms, +M, -M, -theta-terms] (M = 1.5*2^23 = 49152*256,
            # fp16-exact) round the running sum to an integer mid-matmul and
            # leave -frac(q) directly in PSUM: the whole magic-number
            # rounding costs zero DVE ops.  Sin(scale=-2pi) reads the PSUM.
            trig = cpool.tile([KTRIG, QCOLS], f16)
            for ch in range(2):
                cs = slice(512 * ch, 512 * (ch + 1))
                prio = tc.high_priority() if ch == 0 else (
                    contextlib.nullcontext())
                with prio:
                    q = qpool.tile([128, 512], f32, tag="q")
                    pe_chain(nc.tensor.matmul(q[:], pq_v, uq_v[:, cs],
                                              start=True, stop=True))
                    nc.scalar.activation(trig[:, cs], q[0:KTRIG, :],
                                         Act.Sin, scale=-TWO_PI)

            # s matmuls: one 33-col matmul per b; quarter h = b%4 picks the
            # W block, column group g = b//4 picks the trig columns
            szA = szap.tile([128, 512], f32)
            szB = szbp.tile([128, 512], f32)
            szC = szcp.tile([128, 512], f32)
            sz_tiles = (szA, szB, szC)

            def sz_slot(b):
                for wv, (b0, b1) in enumerate(WAVES):
                    if b < b1:
                        return sz_tiles[wv], D * (b - b0)
                raise AssertionError

            big_dummies(BIG_DUMMIES[0])
            for b in range(BLOC):
                if b == 16:
                    big_dummies(BIG_DUMMIES[1])
                g, h = divmod(b, NQ)
                t_sz, off = sz_slot(b)
                pe_chain(nc.tensor.matmul(t_sz[:, off:off + D],
                                          trig[:, 128 * g:128 * (g + 1)],
                                          w_t[:, D * h:D * (h + 1)],
                                          start=True, stop=True))

            # basis = (s + dc) * mask, one fused STT per wave
            bas = cpool.tile([I, BLOC * D], f16)
            for wv, (b0, b1) in enumerate(WAVES):
                ncols = D * (b1 - b0)
                nc.vector.scalar_tensor_tensor(
                    bas[:, D * b0:D * b0 + ncols],
                    in0=sz_tiles[wv][:, 0:ncols], scalar=DC,
                    in1=mk_t[:, D * b0:D * b0 + ncols],
                    op0=Alu.add, op1=Alu.mult)

            # final contraction: accumulate over d (lhsT strided over b-major
            # bas: column b at free index b*D + d)
            big_dummies(BIG_DUMMIES[2])
            basb = bas[:].rearrange("p (b c) -> p c b", c=D)
            y_t = ypool.tile([BLOC, O], f32)
            y_mms = []
            for d in range(D):
                y_mms.append(pe_chain(nc.tensor.matmul(
                    y_t[:], basb[:, d, :], co_t[:, O * d:O * (d + 1)],
                    start=(d == 0), stop=(d == D - 1))))
            # wake the sync DMA queue before the output transfer needs it
            wr1 = cpool.tile([16, 8], f16)
            wr2 = cpool.tile([16, 8], f16)
            for wr_t, dmm in ((wr1, y_mms[12]), (wr2, y_mms[26])):
                wi = nc.sync.dma_start(wr_t[:], co_d.ap()[0:16, 0:8])
                add_dep_helper(wi.ins, dmm.ins, sync=True,
                               reason="keep DMA queue warm for the output")
            y_s = cpool.tile([BLOC, O], f32)
            nc.scalar.copy(y_s[:], y_t[:])
            nc.sync.dma_start(y_d.ap()[:], y_s[:])

    nc.compile()
    return nc


def _host_constants(compression, centers):
    comp = np.asarray(compression, np.float64)
    cent = np.asarray(centers, np.float64)
    assert comp.shape == (D,) and cent.shape == (D,)
    assert np.all(comp == comp[0]), "kernel assumes uniform compression"
    cval = comp[0]

    k = np.arange(1, NK + 1, dtype=np.float64)
    t = (np.pi / A_SUP) * k
    sinc = lambda z: np.sinc(z / np.pi)
    c = sinc(t / 2.0) ** N_ORDER
    for j in range(1, NPROD + 1):
        c = c * sinc(t / (2.0 ** j))

    # per-partition phase constants: theta/2pi split into fp16 hi+lo, and
    # the 1/4-turn offset for cos rows.  Partition map: quarter h occupies
    # [28h, 28h+28) = 14 cos rows then 14 sin rows.
    th = np.zeros(128, np.float64)
    off = np.zeros(128, np.float64)
    feat = t * cval / (2 * np.pi)                    # (NK,)
    for h in range(NQ):
        r = QROWS * h
        th[r:r + NK] = feat
        th[r + NK:r + QROWS] = feat
        off[r:r + NK] = 0.25
    th_h = th.astype(BF16).astype(np.float64)
    th_l = (th - th_h).astype(BF16).astype(np.float64)
    pq = np.zeros((28, 128), np.float64)
    for r0, sgn in ((0, 1.0), (15, -1.0)):
        for h in range(NQ):
            r = QROWS * h
            sel = np.zeros(128)
            sel[r:r + QROWS] = 1.0
            pq[r0 + 3 * h + 0] = sgn * th_h * sel
            pq[r0 + 3 * h + 1] = sgn * th_h * sel
            pq[r0 + 3 * h + 2] = sgn * th_l * sel
        pq[r0 + 12] = sgn * off
    pq[13] = MAGIC_A
    pq[14] = -MAGIC_A

    # feature->series weights; block h zeroes the other quarters' rows.
    # Carries the 1/CO_SCALE output scale; DC term added in the STT.
    phi = np.outer(t * cval, cent)                  # (NK, D)
    s_scale = 1.0 / (A_SUP * CO_SCALE)
    w = np.zeros((KTRIG, NQ * D), np.float64)
    for h in range(NQ):
        blk = D * h
        r = QROWS * h
        w[r:r + NK, blk:blk + D] = c[:, None] * np.cos(phi) * s_scale
        w[r + NK:r + QROWS, blk:blk + D] = c[:, None] * np.sin(phi) * s_scale
    return pq.astype(BF16), w.astype(BF16), cval, cent


def _run(inputs, trace=False, **kw):
    global _PROG
    from concourse.bass_utils import run_bass_kernel_spmd

    if _PROG is None:
        _PROG = _build_program()
    nc = _PROG

    x = np.ascontiguousarray(np.asarray(inputs["x"], np.float32))
    coeffs = np.asarray(inputs["atomic_coeffs"], np.float32)
    pq, w, cval, cent = _host_constants(inputs["compression"],
                                        inputs["centers"])
    co = np.ascontiguousarray(
        (coeffs.transpose(0, 2, 1) * CO_SCALE).astype(BF16).reshape(I, D * O))

    in_maps = []
    for cid in range(NCORES):
        xc = x[cid * BLOC:(cid + 1) * BLOC]              # (32, 128)
        # quarter h holds batch rows b == h (mod 4); col = (b//4)*128 + i
        xq = (xc.reshape(BLOC // NQ, NQ, I).transpose(1, 0, 2)
              .reshape(NQ, QCOLS))
        xh = xq.astype(BF16)
        xl = (xq - xh.astype(np.float32)).astype(BF16)
        ph = np.zeros((28, QCOLS + 128), BF16)
        for r0 in (0, 15):
            for h in range(NQ):
                ph[r0 + 3 * h + 0, :QCOLS] = xh[h]
                ph[r0 + 3 * h + 1, :QCOLS] = xl[h]
                ph[r0 + 3 * h + 2, :QCOLS] = xh[h]
            ph[r0 + 12, :QCOLS] = BF16(1.0)
        ph[13, :QCOLS] = BF16(MAGIC_B)
        ph[14, :QCOLS] = BF16(MAGIC_B)
        ph[:, QCOLS:] = pq
        # support mask [i, b*D + d] = |x[b,i] - c_d| * comp <= a, from exact x
        z = (xc.astype(np.float64)[:, :, None] - cent[None, None, :]) * cval
        mk = np.where(np.abs(z) <= A_SUP, np.uint8(0x38),
                      np.uint8(0))                       # e4m3 1.0 / 0.0
        mk = np.ascontiguousarray(mk.transpose(1, 0, 2).reshape(I, BLOC * D))
        in_maps.append({"ph": ph, "w": w, "mk": mk, "co": co})

    res = run_bass_kernel_spmd(nc, in_maps, core_ids=list(range(NCORES)),
                               trace=trace, **kw)
    # device b index = 4*(b//4) + b%4 = original batch row: no reorder
    y = np.concatenate([res.results[c]["y_s"] for c in range(NCORES)], axis=0)
    return y.astype(np.float32, copy=False), res


def kernel(**inputs):
    y, _ = _run(inputs, trace=False)
    return y


# revision 52
# speedup vs baseline: 1.0511x; 1.0488x over previous
"""Trainium2 Bass kernel for nn_AtomicKANLayer.

Math: y[b,o] = sum_{i,d} fupn((x[b,i]-centers[d])*compression[d]) * coeffs[i,o,d]
with fupn the atomic basis function evaluated via its (truncated) Fourier
series.  With theta_k = t_k*comp and phi_kd = t_k*comp*c_d:

  fupn(z_d) = mask_d * (0.5 + sum_k c_k [cos(theta_k x)cos(phi_kd) +
                                          sin(theta_k x)sin(phi_kd)]) / a

i.e. a dense matmul over per-element trig features cos/sin(theta_k x).  c_15
is exactly 0 and the tail beyond k=14 sums to 3.4e-5, so NK=14 terms suffice;
28 trig rows per quarter * 4 quarters pack the 4096 per-core elements into
112 partitions at 1024 columns, halving every elementwise pass vs a 2-way
split.  The support mask (and the DC 0.5 term) need no device compute at
all: the mask depends only on x, so the host ships it as an fp16 0/1 tensor
and a single fused DVE op per wave does  basis = (s + dc) * mask  straight
out of PSUM.  The final 1/CO_SCALE is folded into the W columns.

Device pipeline per core (data-parallel over batch, 32 rows of B=256 each):
  1. q[p,c] = theta_p/2pi * x + off_p  as an exact split-fp16 matmul
     (th_h*xh + th_h*xl + th_l*xh, 13-row contraction, fp32 PSUM)     [PE]
  2. f = q - round(q) (fp32 magic-constant trick: ts add/sub, tt sub) [DVE]
  3. trig[0:112] = Sin(2pi f) -> fp16 (ACT free scale)                [ACT]
  4. s = trig-chunk.T @ W_h (fp16 K=112, 33 cols) per b               [PE]
  5. basis = (s + dc) * mask: fused STT per wave                      [DVE]
  6. y = sum_d basis_d.T @ coeffs_d (33 fp16 matmuls, fp32 PSUM)      [PE]
  7. y PSUM -> SBUF copy (ACT), DMA out                               [ACT]
All input DMAs ride one sync-HWDGE FIFO (the DMA engines serve the
first-ringing queue until it drains, so cross-queue splits starve someone):
ph first (q gate), then W (s-matmul gate), the fp8 mask, and the 2.1MB
coefficient stream in 4 d-chunks consumed in order by step 6.  Warm-up
matmuls on const/loaded tiles keep the tensor clock ramping (0.65/1.2/2.4
GHz p-states) through every pipeline stall, and two tiny warmer DMAs keep
the queue hot for the output transfer.
"""
import sys

sys.path.insert(0, "/opt/trn_rl_repo")

import numpy as np

BF16 = np.float16  # half precision for PE operands (fp32 PSUM accumulate)
B, I, O, D = 256, 128, 256, 33
NCORES = 8
BLOC = B // NCORES          # 32 batch rows per core
ROWS = BLOC * I             # 4096 flattened (b, i) elements per core
NQ = 4                      # partition-packing quarters
QCOLS = ROWS // NQ          # 1024 columns per quarter
NK = 14                     # Fourier terms kept (c_15 == 0 exactly)
QROWS = 2 * NK              # cos/sin rows per quarter
KTRIG = NQ * QROWS          # 112 trig partitions
N_ORDER, NPROD = 1, 10
A_SUP = (N_ORDER + 2) / 2.0  # support half-width a = 1.5
MAGIC_A, MAGIC_B = 49152.0, 256.0  # fp16-exact pair, product 1.5*2^23
TWO_PI = float(2 * np.pi)
CO_DMA_CHUNKS = 4
CO_SCALE = 256.0            # lift fp16 coeffs out of subnormal range
DC = float(0.5 / (A_SUP * CO_SCALE))  # series DC term, added in the STT
# STT waves: (b0, b1); each wave lives in its own PSUM bank (<=15 b's)
WAVES = ((0, 15), (15, 26), (26, 32))
# PE warm-up dummies: the tensor clock ramps 0.65->1.2->2.4GHz with
# sustained execution; tiny matmuls fill every idle window so the final
# contraction runs fully ramped.  (pre-ph, post-q, mid-wave, pre-y)
DUMMIES = (9,)
BIG_DUMMIES = (4, 2, 6)   # post-q, mid-wave, pre-y (y-shaped, ramp the clock)

_PROG = None


def _build_program():
    import concourse.bacc as bacc
    import concourse.tile as tile
    from concourse import mybir

    f32 = mybir.dt.float32
    f16 = mybir.dt.float16
    f8 = mybir.dt.float8e4
    Alu = mybir.AluOpType
    Act = mybir.ActivationFunctionType

    nc = bacc.Bacc("TRN2", target_bir_lowering=False, debug=False,
                   num_devices=NCORES)
    ph_d = nc.dram_tensor("ph", [28, QCOLS + 128], f16, kind="ExternalInput")
    w_d = nc.dram_tensor("w", [KTRIG, NQ * D], f16, kind="ExternalInput")
    mk_d = nc.dram_tensor("mk", [I, BLOC * D], f8, kind="ExternalInput")
    co_d = nc.dram_tensor("co", [I, D * O], f16, kind="ExternalInput")
    y_d = nc.dram_tensor("y_s", [BLOC, O], f32, kind="ExternalOutput")

    with tile.TileContext(nc) as tc:
        with (
            tc.tile_pool(name="const", bufs=1) as cpool,
            tc.tile_pool(name="work", bufs=2) as wpool,
            tc.tile_pool(name="qp", bufs=2, space="PSUM") as qpool,
            tc.tile_pool(name="sza", bufs=1, space="PSUM") as szap,
            tc.tile_pool(name="szb", bufs=1, space="PSUM") as szbp,
            tc.tile_pool(name="szc", bufs=1, space="PSUM") as szcp,
            tc.tile_pool(name="yp", bufs=1, space="PSUM") as ypool,
            tc.tile_pool(name="dum", bufs=1, space="PSUM") as dumpool,
        ):
            from concourse.tile_rust import add_dep_helper

            # one FIFO: ph (padded to 16 rows -- one descriptor per DMA
            # engine for the fastest completion credit), then w, mask, and
            # the coefficient chunks
            ph_t = cpool.tile([28, QCOLS + 128], f16)
            with tc.high_priority():
                nc.scalar.dma_start(ph_t[:], ph_d.ap()[:])
            co_t = cpool.tile([I, D * O], f16)
            dper = (D + CO_DMA_CHUNKS - 1) // CO_DMA_CHUNKS

            def co_dma(c):
                d0, d1 = c * dper, min(D, (c + 1) * dper)
                nc.sync.dma_start(co_t[:, O * d0:O * d1],
                                  co_d.ap()[:, O * d0:O * d1])

            w_t = cpool.tile([KTRIG, NQ * D], f16)
            nc.sync.dma_start(w_t[:], w_d.ap()[:])
            mk_t = cpool.tile([I, BLOC * D], f8)
            nc.sync.dma_start(mk_t[:], mk_d.ap()[:])
            for c in range(CO_DMA_CHUNKS):
                co_dma(c)

            uq_v = ph_t[:, 0:QCOLS]
            pq_v = ph_t[:, QCOLS:QCOLS + 128]

            import contextlib

            # PE warm-up dummies: back-to-back 1x1 matmuls on the framework
            # const AP keep the tensor clock ramping through every stall
            cb = nc.const_aps.aps[(mybir.dt.bfloat16, 1.0)]
            cbl = nc.const_aps.tensor(1.0, (128, 128), mybir.dt.bfloat16)
            cbr = nc.const_aps.tensor(1.0, (128, O), mybir.dt.bfloat16)
            dumc = dumpool.tile([128, O], f32)
            pe_last = [None]

            def pe_chain(ins_obj):
                ins = getattr(ins_obj, "ins", ins_obj)
                if pe_last[0] is not None:
                    add_dep_helper(ins, pe_last[0], sync=False,
                                   reason="keep PE in emission order")
                pe_last[0] = ins
                return ins_obj

            # y-shaped dummies (256-col streams off the mask tile): the
            # clock governor ramps on sustained HIGH-utilization matmuls,
            # not on tiny ones, so these pre-heat the final contraction
            dumb = dumpool.tile([32, O], f32)

            def big_dummies(n):
                for _ in range(n):
                    pe_chain(nc.tensor.matmul(dumb[:], ph_t[:, 0:32],
                                              ph_t[:, 0:O],
                                              start=True, stop=True))

            # full-power warm-up before any input lands: broadcast const
            # operands, full 128x128 array, 256-col streams
            for _ in range(DUMMIES[0]):
                pe_chain(nc.tensor.matmul(dumc[:], cbl, cbr,
                                          start=True, stop=True))

            # phase matmul, two 512-column chunks.  The PE accumulates the
            # K dot product sequentially in row order in fp32, so rows
            # [theta-terms, +M, -M, -theta-terms] (M = 1.5*2^23 = 49152*256,
            # fp16-exact) round the running sum to an integer mid-matmul and
            # leave -frac(q) directly in PSUM: the whole magic-number
            # rounding costs zero DVE ops.  Sin(scale=-2pi) reads the PSUM.
            trig = cpool.tile([KTRIG, QCOLS], f16)
            for ch in range(2):
                cs = slice(512 * ch, 512 * (ch + 1))
                prio = tc.high_priority() if ch == 0 else (
                    contextlib.nullcontext())
                with prio:
                    q = qpool.tile([128, 512], f32, tag="q")
                    pe_chain(nc.tensor.matmul(q[:], pq_v, uq_v[:, cs],
                                              start=True, stop=True))
                    nc.scalar.activation(trig[:, cs], q[0:KTRIG, :],
                                         Act.Sin, scale=-TWO_PI)

            # s matmuls: one 33-col matmul per b; quarter h = b%4 picks the
            # W block, column group g = b//4 picks the trig columns
            szA = szap.tile([128, 512], f32)
            szB = szbp.tile([128, 512], f32)
            szC = szcp.tile([128, 512], f32)
            sz_tiles = (szA, szB, szC)

            def sz_slot(b):
                for wv, (b0, b1) in enumerate(WAVES):
                    if b < b1:
                        return sz_tiles[wv], D * (b - b0)
                raise AssertionError

            big_dummies(BIG_DUMMIES[0])
            for b in range(BLOC):
                if b == 16:
                    big_dummies(BIG_DUMMIES[1])
                g, h = divmod(b, NQ)
                t_sz, off = sz_slot(b)
                pe_chain(nc.tensor.matmul(t_sz[:, off:off + D],
                                          trig[:, 128 * g:128 * (g + 1)],
                                          w_t[:, D * h:D * (h + 1)],
                                          start=True, stop=True))

            # basis = (s + dc) * mask, one fused STT per wave
            bas = cpool.tile([I, BLOC * D], f16)
            for wv, (b0, b1) in enumerate(WAVES):
                ncols = D * (b1 - b0)
                nc.vector.scalar_tensor_tensor(
                    bas[:, D * b0:D * b0 + ncols],
                    in0=sz_tiles[wv][:, 0:ncols], scalar=DC,
                    in1=mk_t[:, D * b0:D * b0 + ncols],
                    op0=Alu.add, op1=Alu.mult)

            # final contraction: accumulate over d (lhsT strided over b-major
            # bas: column b at free index b*D + d)
            big_dummies(BIG_DUMMIES[2])
            basb = bas[:].rearrange("p (b c) -> p c b", c=D)
            y_t = ypool.tile([BLOC, O], f32)
            y_mms = []
            for d in range(D):
                y_mms.append(pe_chain(nc.tensor.matmul(
                    y_t[:], basb[:, d, :], co_t[:, O * d:O * (d + 1)],
                    start=(d == 0), stop=(d == D - 1))))
            # wake the sync DMA queue before the output transfer needs it
            wr1 = cpool.tile([16, 8], f16)
            wr2 = cpool.tile([16, 8], f16)
            for wr_t, dmm in ((wr1, y_mms[12]), (wr2, y_mms[26])):
                wi = nc.sync.dma_start(wr_t[:], co_d.ap()[0:16, 0:8])
                add_dep_helper(wi.ins, dmm.ins, sync=True,
                               reason="keep DMA queue warm for the output")
            y_s = cpool.tile([BLOC, O], f32)
            nc.scalar.copy(y_s[:], y_t[:])
            nc.sync.dma_start(y_d.ap()[:], y_s[:])

    nc.compile()
    return nc


def _host_constants(compression, centers):
    comp = np.asarray(compression, np.float64)
    cent = np.asarray(centers, np.float64)
    assert comp.shape == (D,) and cent.shape == (D,)
    assert np.all(comp == comp[0]), "kernel assumes uniform compression"
    cval = comp[0]

    k = np.arange(1, NK + 1, dtype=np.float64)
    t = (np.pi / A_SUP) * k
    sinc = lambda z: np.sinc(z / np.pi)
    c = sinc(t / 2.0) ** N_ORDER
    for j in range(1, NPROD + 1):
        c = c * sinc(t / (2.0 ** j))

    # per-partition phase constants: theta/2pi split into fp16 hi+lo, and
    # the 1/4-turn offset for cos rows.  Partition map: quarter h occupies
    # [28h, 28h+28) = 14 cos rows then 14 sin rows.
    th = np.zeros(128, np.float64)
    off = np.zeros(128, np.float64)
    feat = t * cval / (2 * np.pi)                    # (NK,)
    for h in range(NQ):
        r = QROWS * h
        th[r:r + NK] = feat
        th[r + NK:r + QROWS] = feat
        off[r:r + NK] = 0.25
    th_h = th.astype(BF16).astype(np.float64)
    th_l = (th - th_h).astype(BF16).astype(np.float64)
    pq = np.zeros((28, 128), np.float64)
    for r0, sgn in ((0, 1.0), (15, -1.0)):
        for h in range(NQ):
            r = QROWS * h
            sel = np.zeros(128)
            sel[r:r + QROWS] = 1.0
            pq[r0 + 3 * h + 0] = sgn * th_h * sel
            pq[r0 + 3 * h + 1] = sgn * th_h * sel
            pq[r0 + 3 * h + 2] = sgn * th_l * sel
        pq[r0 + 12] = sgn * off
    pq[13] = MAGIC_A
    pq[14] = -MAGIC_A

    # feature->series weights; block h zeroes the other quarters' rows.
    # Carries the 1/CO_SCALE output scale; DC term added in the STT.
    phi = np.outer(t * cval, cent)                  # (NK, D)
    s_scale = 1.0 / (A_SUP * CO_SCALE)
    w = np.zeros((KTRIG, NQ * D), np.float64)
    for h in range(NQ):
        blk = D * h
        r = QROWS * h
        w[r:r + NK, blk:blk + D] = c[:, None] * np.cos(phi) * s_scale
        w[r + NK:r + QROWS, blk:blk + D] = c[:, None] * np.sin(phi) * s_scale
    return pq.astype(BF16), w.astype(BF16), cval, cent


def _run(inputs, trace=False, **kw):
    global _PROG
    from concourse.bass_utils import run_bass_kernel_spmd

    if _PROG is None:
        _PROG = _build_program()
    nc = _PROG

    x = np.ascontiguousarray(np.asarray(inputs["x"], np.float32))
    coeffs = np.asarray(inputs["atomic_coeffs"], np.float32)
    pq, w, cval, cent = _host_constants(inputs["compression"],
                                        inputs["centers"])
    co = np.ascontiguousarray(
        (coeffs.transpose(0, 2, 1) * CO_SCALE).astype(BF16).reshape(I, D * O))

    in_maps = []
    for cid in range(NCORES):
        xc = x[cid * BLOC:(cid + 1) * BLOC]              # (32, 128)
        # quarter h holds batch rows b == h (mod 4); col = (b//4)*128 + i
        xq = (xc.reshape(BLOC // NQ, NQ, I).transpose(1, 0, 2)
              .reshape(NQ, QCOLS))
        xh = xq.astype(BF16)
        xl = (xq - xh.astype(np.float32)).astype(BF16)
        ph = np.zeros((28, QCOLS + 128), BF16)
        for r0 in (0, 15):
            for h in range(NQ):
                ph[r0 + 3 * h + 0, :QCOLS] = xh[h]
                ph[r0 + 3 * h + 1, :QCOLS] = xl[h]
                ph[r0 + 3 * h + 2, :QCOLS] = xh[h]
            ph[r0 + 12, :QCOLS] = BF16(1.0)
        ph[13, :QCOLS] = BF16(MAGIC_B)
        ph[14, :QCOLS] = BF16(MAGIC_B)
        ph[:, QCOLS:] = pq
        # support mask [i, b*D + d] = |x[b,i] - c_d| * comp <= a, from exact x
        z = (xc.astype(np.float64)[:, :, None] - cent[None, None, :]) * cval
        mk = np.where(np.abs(z) <= A_SUP, np.uint8(0x38),
                      np.uint8(0))                       # e4m3 1.0 / 0.0
        mk = np.ascontiguousarray(mk.transpose(1, 0, 2).reshape(I, BLOC * D))
        in_maps.append({"ph": ph, "w": w, "mk": mk, "co": co})

    res = run_bass_kernel_spmd(nc, in_maps, core_ids=list(range(NCORES)),
                               trace=trace, **kw)
    # device b index = 4*(b//4) + b%4 = original batch row: no reorder
    y = np.concatenate([res.results[c]["y_s"] for c in range(NCORES)], axis=0)
    return y.astype(np.float32, copy=False), res


def kernel(**inputs):
    y, _ = _run(inputs, trace=False)
    return y


# revision 53
# speedup vs baseline: 1.0550x; 1.0037x over previous
"""Trainium2 Bass kernel for nn_AtomicKANLayer.

Math: y[b,o] = sum_{i,d} fupn((x[b,i]-centers[d])*compression[d]) * coeffs[i,o,d]
with fupn the atomic basis function evaluated via its (truncated) Fourier
series.  With theta_k = t_k*comp and phi_kd = t_k*comp*c_d:

  fupn(z_d) = mask_d * (0.5 + sum_k c_k [cos(theta_k x)cos(phi_kd) +
                                          sin(theta_k x)sin(phi_kd)]) / a

i.e. a dense matmul over per-element trig features cos/sin(theta_k x).  c_15
is exactly 0 and the tail beyond k=14 sums to 3.4e-5, so NK=14 terms suffice;
28 trig rows per quarter * 4 quarters pack the 4096 per-core elements into
112 partitions at 1024 columns, halving every elementwise pass vs a 2-way
split.  The support mask (and the DC 0.5 term) need no device compute at
all: the mask depends only on x, so the host ships it as an fp16 0/1 tensor
and a single fused DVE op per wave does  basis = (s + dc) * mask  straight
out of PSUM.  The final 1/CO_SCALE is folded into the W columns.

Device pipeline per core (data-parallel over batch, 32 rows of B=256 each):
  1. q[p,c] = theta_p/2pi * x + off_p  as an exact split-fp16 matmul
     (th_h*xh + th_h*xl + th_l*xh, 13-row contraction, fp32 PSUM)     [PE]
  2. f = q - round(q) (fp32 magic-constant trick: ts add/sub, tt sub) [DVE]
  3. trig[0:112] = Sin(2pi f) -> fp16 (ACT free scale)                [ACT]
  4. s = trig-chunk.T @ W_h (fp16 K=112, 33 cols) per b               [PE]
  5. basis = (s + dc) * mask: fused STT per wave                      [DVE]
  6. y = sum_d basis_d.T @ coeffs_d (33 fp16 matmuls, fp32 PSUM)      [PE]
  7. y PSUM -> SBUF copy (ACT), DMA out                               [ACT]
All input DMAs ride one sync-HWDGE FIFO (the DMA engines serve the
first-ringing queue until it drains, so cross-queue splits starve someone):
ph first (q gate), then W (s-matmul gate), the fp8 mask, and the 2.1MB
coefficient stream in 4 d-chunks consumed in order by step 6.  Warm-up
matmuls on const/loaded tiles keep the tensor clock ramping (0.65/1.2/2.4
GHz p-states) through every pipeline stall, and two tiny warmer DMAs keep
the queue hot for the output transfer.
"""
import sys

sys.path.insert(0, "/opt/trn_rl_repo")

import numpy as np

BF16 = np.float16  # half precision for PE operands (fp32 PSUM accumulate)
B, I, O, D = 256, 128, 256, 33
NCORES = 8
BLOC = B // NCORES          # 32 batch rows per core
ROWS = BLOC * I             # 4096 flattened (b, i) elements per core
NQ = 4                      # partition-packing quarters
QCOLS = ROWS // NQ          # 1024 columns per quarter
NK = 14                     # Fourier terms kept (c_15 == 0 exactly)
QROWS = 2 * NK              # cos/sin rows per quarter
KTRIG = NQ * QROWS          # 112 trig partitions
N_ORDER, NPROD = 1, 10
A_SUP = (N_ORDER + 2) / 2.0  # support half-width a = 1.5
MAGIC_A, MAGIC_B = 49152.0, 256.0  # fp16-exact pair, product 1.5*2^23
TWO_PI = float(2 * np.pi)
CO_DMA_CHUNKS = 4
CO_SCALE = 256.0            # lift fp16 coeffs out of subnormal range
DC = float(0.5 / (A_SUP * CO_SCALE))  # series DC term, added in the STT
# STT waves: (b0, b1); each wave lives in its own PSUM bank (<=15 b's)
WAVES = ((0, 15), (15, 26), (26, 32))
# PE warm-up dummies: the tensor clock ramps 0.65->1.2->2.4GHz with
# sustained execution; tiny matmuls fill every idle window so the final
# contraction runs fully ramped.  (pre-ph, post-q, mid-wave, pre-y)
DUMMIES = (9,)
BIG_DUMMIES = (3, 2, 4)   # post-q, mid-wave, pre-y (y-shaped, ramp the clock)

_PROG = None


def _build_program():
    import concourse.bacc as bacc
    import concourse.tile as tile
    from concourse import mybir

    f32 = mybir.dt.float32
    f16 = mybir.dt.float16
    f8 = mybir.dt.float8e4
    Alu = mybir.AluOpType
    Act = mybir.ActivationFunctionType

    nc = bacc.Bacc("TRN2", target_bir_lowering=False, debug=False,
                   num_devices=NCORES)
    ph_d = nc.dram_tensor("ph", [28, QCOLS + 128], f16, kind="ExternalInput")
    w_d = nc.dram_tensor("w", [KTRIG, NQ * D], f16, kind="ExternalInput")
    mk_d = nc.dram_tensor("mk", [I, BLOC * D], f8, kind="ExternalInput")
    co_d = nc.dram_tensor("co", [I, D * O], f16, kind="ExternalInput")
    y_d = nc.dram_tensor("y_s", [BLOC, O], f32, kind="ExternalOutput")

    with tile.TileContext(nc) as tc:
        with (
            tc.tile_pool(name="const", bufs=1) as cpool,
            tc.tile_pool(name="work", bufs=2) as wpool,
            tc.tile_pool(name="qp", bufs=2, space="PSUM") as qpool,
            tc.tile_pool(name="sza", bufs=1, space="PSUM") as szap,
            tc.tile_pool(name="szb", bufs=1, space="PSUM") as szbp,
            tc.tile_pool(name="szc", bufs=1, space="PSUM") as szcp,
            tc.tile_pool(name="yp", bufs=1, space="PSUM") as ypool,
            tc.tile_pool(name="dum", bufs=1, space="PSUM") as dumpool,
        ):
            from concourse.tile_rust import add_dep_helper

            # one FIFO: ph (padded to 16 rows -- one descriptor per DMA
            # engine for the fastest completion credit), then w, mask, and
            # the coefficient chunks
            ph_t = cpool.tile([28, QCOLS + 128], f16)
            with tc.high_priority():
                nc.scalar.dma_start(ph_t[:], ph_d.ap()[:])
            co_t = cpool.tile([I, D * O], f16)
            dper = (D + CO_DMA_CHUNKS - 1) // CO_DMA_CHUNKS

            def co_dma(c):
                d0, d1 = c * dper, min(D, (c + 1) * dper)
                nc.sync.dma_start(co_t[:, O * d0:O * d1],
                                  co_d.ap()[:, O * d0:O * d1])

            w_t = cpool.tile([KTRIG, NQ * D], f16)
            nc.sync.dma_start(w_t[:], w_d.ap()[:])
            mk_t = cpool.tile([I, BLOC * D], f8)
            nc.sync.dma_start(mk_t[:], mk_d.ap()[:])
            for c in range(CO_DMA_CHUNKS):
                co_dma(c)

            uq_v = ph_t[:, 0:QCOLS]
            pq_v = ph_t[:, QCOLS:QCOLS + 128]

            import contextlib

            # PE warm-up dummies: back-to-back 1x1 matmuls on the framework
            # const AP keep the tensor clock ramping through every stall
            cb = nc.const_aps.aps[(mybir.dt.bfloat16, 1.0)]
            cbl = nc.const_aps.tensor(1.0, (128, 128), mybir.dt.bfloat16)
            cbr = nc.const_aps.tensor(1.0, (128, O), mybir.dt.bfloat16)
            dumc = dumpool.tile([128, O], f32)
            pe_last = [None]

            def pe_chain(ins_obj):
                ins = getattr(ins_obj, "ins", ins_obj)
                if pe_last[0] is not None:
                    add_dep_helper(ins, pe_last[0], sync=False,
                                   reason="keep PE in emission order")
                pe_last[0] = ins
                return ins_obj

            # y-shaped dummies (256-col streams off the mask tile): the
            # clock governor ramps on sustained HIGH-utilization matmuls,
            # not on tiny ones, so these pre-heat the final contraction
            dumb = dumpool.tile([32, O], f32)

            def big_dummies(n):
                for _ in range(n):
                    pe_chain(nc.tensor.matmul(dumb[:], ph_t[:, 0:32],
                                              ph_t[:, 0:O],
                                              start=True, stop=True))

            # full-power warm-up before any input lands: broadcast const
            # operands, full 128x128 array, 256-col streams
            for _ in range(DUMMIES[0]):
                pe_chain(nc.tensor.matmul(dumc[:], cbl, cbr,
                                          start=True, stop=True))

            # phase matmul, two 512-column chunks.  The PE accumulates the
            # K dot product sequentially in row order in fp32, so rows
            # [theta-terms, +M, -M, -theta-terms] (M = 1.5*2^23 = 49152*256,
            # fp16-exact) round the running sum to an integer mid-matmul and
            # leave -frac(q) directly in PSUM: the whole magic-number
            # rounding costs zero DVE ops.  Sin(scale=-2pi) reads the PSUM.
            trig = cpool.tile([KTRIG, QCOLS], f16)
            for ch in range(2):
                cs = slice(512 * ch, 512 * (ch + 1))
                prio = tc.high_priority() if ch == 0 else (
                    contextlib.nullcontext())
                with prio:
                    q = qpool.tile([128, 512], f32, tag="q")
                    pe_chain(nc.tensor.matmul(q[:], pq_v, uq_v[:, cs],
                                              start=True, stop=True))
                    nc.scalar.activation(trig[:, cs], q[0:KTRIG, :],
                                         Act.Sin, scale=-TWO_PI)

            # s matmuls: one 33-col matmul per b; quarter h = b%4 picks the
            # W block, column group g = b//4 picks the trig columns
            szA = szap.tile([128, 512], f32)
            szB = szbp.tile([128, 512], f32)
            szC = szcp.tile([128, 512], f32)
            sz_tiles = (szA, szB, szC)

            def sz_slot(b):
                for wv, (b0, b1) in enumerate(WAVES):
                    if b < b1:
                        return sz_tiles[wv], D * (b - b0)
                raise AssertionError

            big_dummies(BIG_DUMMIES[0])
            for b in range(BLOC):
                if b == 16:
                    big_dummies(BIG_DUMMIES[1])
                g, h = divmod(b, NQ)
                t_sz, off = sz_slot(b)
                pe_chain(nc.tensor.matmul(t_sz[:, off:off + D],
                                          trig[:, 128 * g:128 * (g + 1)],
                                          w_t[:, D * h:D * (h + 1)],
                                          start=True, stop=True))

            # basis = (s + dc) * mask, one fused STT per wave
            bas = cpool.tile([I, BLOC * D], f16)
            for wv, (b0, b1) in enumerate(WAVES):
                ncols = D * (b1 - b0)
                nc.vector.scalar_tensor_tensor(
                    bas[:, D * b0:D * b0 + ncols],
                    in0=sz_tiles[wv][:, 0:ncols], scalar=DC,
                    in1=mk_t[:, D * b0:D * b0 + ncols],
                    op0=Alu.add, op1=Alu.mult)

            # final contraction: accumulate over d (lhsT strided over b-major
            # bas: column b at free index b*D + d)
            big_dummies(BIG_DUMMIES[2])
            basb = bas[:].rearrange("p (b c) -> p c b", c=D)
            y_t = ypool.tile([BLOC, O], f32)
            y_mms = []
            for d in range(D):
                y_mms.append(pe_chain(nc.tensor.matmul(
                    y_t[:], basb[:, d, :], co_t[:, O * d:O * (d + 1)],
                    start=(d == 0), stop=(d == D - 1))))
            # wake the sync DMA queue before the output transfer needs it
            wr1 = cpool.tile([16, 8], f16)
            wr2 = cpool.tile([16, 8], f16)
            for wr_t, dmm in ((wr1, y_mms[12]), (wr2, y_mms[26])):
                wi = nc.sync.dma_start(wr_t[:], co_d.ap()[0:16, 0:8])
                add_dep_helper(wi.ins, dmm.ins, sync=True,
                               reason="keep DMA queue warm for the output")
            y_s = cpool.tile([BLOC, O], f32)
            nc.scalar.copy(y_s[:], y_t[:])
            nc.sync.dma_start(y_d.ap()[:], y_s[:])

    nc.compile()
    return nc


def _host_constants(compression, centers):
    comp = np.asarray(compression, np.float64)
    cent = np.asarray(centers, np.float64)
    assert comp.shape == (D,) and cent.shape == (D,)
    assert np.all(comp == comp[0]), "kernel assumes uniform compression"
    cval = comp[0]

    k = np.arange(1, NK + 1, dtype=np.float64)
    t = (np.pi / A_SUP) * k
    sinc = lambda z: np.sinc(z / np.pi)
    c = sinc(t / 2.0) ** N_ORDER
    for j in range(1, NPROD + 1):
        c = c * sinc(t / (2.0 ** j))

    # per-partition phase constants: theta/2pi split into fp16 hi+lo, and
    # the 1/4-turn offset for cos rows.  Partition map: quarter h occupies
    # [28h, 28h+28) = 14 cos rows then 14 sin rows.
    th = np.zeros(128, np.float64)
    off = np.zeros(128, np.float64)
    feat = t * cval / (2 * np.pi)                    # (NK,)
    for h in range(NQ):
        r = QROWS * h
        th[r:r + NK] = feat
        th[r + NK:r + QROWS] = feat
        off[r:r + NK] = 0.25
    th_h = th.astype(BF16).astype(np.float64)
    th_l = (th - th_h).astype(BF16).astype(np.float64)
    pq = np.zeros((28, 128), np.float64)
    for r0, sgn in ((0, 1.0), (15, -1.0)):
        for h in range(NQ):
            r = QROWS * h
            sel = np.zeros(128)
            sel[r:r + QROWS] = 1.0
            pq[r0 + 3 * h + 0] = sgn * th_h * sel
            pq[r0 + 3 * h + 1] = sgn * th_h * sel
            pq[r0 + 3 * h + 2] = sgn * th_l * sel
        pq[r0 + 12] = sgn * off
    pq[13] = MAGIC_A
    pq[14] = -MAGIC_A

    # feature->series weights; block h zeroes the other quarters' rows.
    # Carries the 1/CO_SCALE output scale; DC term added in the STT.
    phi = np.outer(t * cval, cent)                  # (NK, D)
    s_scale = 1.0 / (A_SUP * CO_SCALE)
    w = np.zeros((KTRIG, NQ * D), np.float64)
    for h in range(NQ):
        blk = D * h
        r = QROWS * h
        w[r:r + NK, blk:blk + D] = c[:, None] * np.cos(phi) * s_scale
        w[r + NK:r + QROWS, blk:blk + D] = c[:, None] * np.sin(phi) * s_scale
    return pq.astype(BF16), w.astype(BF16), cval, cent


def _run(inputs, trace=False, **kw):
    global _PROG
    from concourse.bass_utils import run_bass_kernel_spmd

    if _PROG is None:
        _PROG = _build_program()
    nc = _PROG

    x = np.ascontiguousarray(np.asarray(inputs["x"], np.float32))
    coeffs = np.asarray(inputs["atomic_coeffs"], np.float32)
    pq, w, cval, cent = _host_constants(inputs["compression"],
                                        inputs["centers"])
    co = np.ascontiguousarray(
        (coeffs.transpose(0, 2, 1) * CO_SCALE).astype(BF16).reshape(I, D * O))

    in_maps = []
    for cid in range(NCORES):
        xc = x[cid * BLOC:(cid + 1) * BLOC]              # (32, 128)
        # quarter h holds batch rows b == h (mod 4); col = (b//4)*128 + i
        xq = (xc.reshape(BLOC // NQ, NQ, I).transpose(1, 0, 2)
              .reshape(NQ, QCOLS))
        xh = xq.astype(BF16)
        xl = (xq - xh.astype(np.float32)).astype(BF16)
        ph = np.zeros((28, QCOLS + 128), BF16)
        for r0 in (0, 15):
            for h in range(NQ):
                ph[r0 + 3 * h + 0, :QCOLS] = xh[h]
                ph[r0 + 3 * h + 1, :QCOLS] = xl[h]
                ph[r0 + 3 * h + 2, :QCOLS] = xh[h]
            ph[r0 + 12, :QCOLS] = BF16(1.0)
        ph[13, :QCOLS] = BF16(MAGIC_B)
        ph[14, :QCOLS] = BF16(MAGIC_B)
        ph[:, QCOLS:] = pq
        # support mask [i, b*D + d] = |x[b,i] - c_d| * comp <= a, from exact x
        z = (xc.astype(np.float64)[:, :, None] - cent[None, None, :]) * cval
        mk = np.where(np.abs(z) <= A_SUP, np.uint8(0x38),
                      np.uint8(0))                       # e4m3 1.0 / 0.0
        mk = np.ascontiguousarray(mk.transpose(1, 0, 2).reshape(I, BLOC * D))
        in_maps.append({"ph": ph, "w": w, "mk": mk, "co": co})

    res = run_bass_kernel_spmd(nc, in_maps, core_ids=list(range(NCORES)),
                               trace=trace, **kw)
    # device b index = 4*(b//4) + b%4 = original batch row: no reorder
    y = np.concatenate([res.results[c]["y_s"] for c in range(NCORES)], axis=0)
    return y.astype(np.float32, copy=False), res


def kernel(**inputs):
    y, _ = _run(inputs, trace=False)
    return y
